# revision 7
# baseline (speedup 1.0000x reference)
"""Trainium2 Bass kernel for nn_Comm_OUT (MTRNN -> Ted_Conv1d -> proj -> comm mask).

Data-parallel over N = E*S = 2048 sequences across 8 NeuronCores (256 each).
I/O-lean + stream-pipelined variant: the harness-visible cost is dominated by
host<->device traffic, so
  - x ships as bf16 in a PE-ready layout, split into 4 n-quarters per core so
    compute (and the output stream) starts before the full x has streamed in,
  - all weights ship as TWO bf16 blobs sharded 1/8th per core and AllGather'd
    on-device (blob A: Wx+Wh needed first; blob B: conv+out weights),
  - the output ships as bf16 and is upcast on host.
Compute per core per n-quarter of 64 rows (all matmuls bf16, f32 PSUM):
  phase 0: xw = x @ Wx                      (transposed layout: H on partitions)
  phase 1+2 interleaved: 32-step MTRNN h = tanh(xw + h @ Wh + b) writing a
           16-slot ring of hidden states, with the 4 parallel convs
           (k=1,3,5,7, reflect padding, 8 l-positions per PSUM tile) consuming
           the ring as shifted matmuls; PReLU fused into PSUM eviction
  phase 3: projection to C=64 logits per position
  phase 4: comm mask = "no end token (argmax==0) strictly before l"
  outputs DMA'd per (quarter, 8-l tile) so the write stream overlaps the rest.
"""

import numpy as np
import ml_dtypes

import concourse.bass as bass
import concourse.mybir as mybir
from concourse.tile import TileContext
from concourse.bass_utils import run_bass_kernel_spmd

F32 = mybir.dt.float32
BF16 = mybir.dt.bfloat16
AF = mybir.ActivationFunctionType
ALU = mybir.AluOpType
NPBF16 = ml_dtypes.bfloat16

E, S, L, H, D_IN, C = 32, 64, 32, 512, 1536, 64
N = E * S
NCORES = 8
NC_N = N // NCORES          # 256 rows per core
NQ = 4                      # n-quarters per core
QN = NC_N // NQ             # 64 rows per quarter
HC = H // 128               # 4 H chunks
DC = D_IN // 128            # 12 D_IN chunks
TL = 8                      # output-l positions per conv PSUM tile
RING = 16                   # hidden-state ring depth (l modulo RING)
KS = [1, 3, 5, 7]

# ---- weight blob column maps (bf16) ----
# blob A [128, CBA]: wx (d*4+m)*128, wh 6144+(kc*4+m)*128
WXO = 0
WHO = 6144
CBA = 8192
# blob B [128, CBB]: wc{k} at WCB[k] + (dk*4+kc)*128, wo at WOO + kc*64
WCB = {1: 0, 3: 512, 5: 2048, 7: 4608}
WOO = 8192
CBB = 8448
SHROWS = 128 // NCORES      # blob partition rows shipped per core

_uid = [0]


def _split_excess_waits(nc, limit=1):
    """walrus in this toolchain accepts at most one sem-wait per instruction;
    move excess waits onto same-engine no-ops inserted just before."""
    for f in nc.m.functions:
        for bb in f.blocks:
            insts = bb.instructions
            i = 0
            while i < len(insts):
                inst = insts[i]
                si = inst.sync_info
                waits = list(si.on_wait) if si and si.on_wait else []
                if len(waits) > limit:
                    excess, keep = waits[:-limit], waits[-limit:]
                    inst.sync_info = mybir.SyncInfo(
                        on_wait=keep, on_update=list(si.on_update or []))
                    pos = i
                    for j in range(0, len(excess), limit):
                        _uid[0] += 1
                        nop = mybir.InstNoOp(
                            name=f"I-waitsplit-{_uid[0]}", ins=[], outs=[])
                        nop.engine = inst.engine
                        nop.bass_nofuse = True
                        nop.sync_info = mybir.SyncInfo(
                            on_wait=excess[j:j + limit], on_update=[])
                        insts.insert(pos, nop)
                        nc.register_instruction(nop, overwrite=True)
                        pos += 1
                        i += 1
                i += 1
            bb.instructions = insts


def _reflect(i):
    if i < 0:
        return -i
    if i > L - 1:
        return 2 * (L - 1) - i
    return i


def _conv_mm_plan():
    """Per (ltile, conv): ordered list of (dk, kc, slot0, n_l, out_j) matmuls
    over the hidden-state ring. Runs split on non-contiguity (reflection) and
    on ring wrap. The identity tap (offset 0) goes first and never wraps
    (l0 multiple of TL, RING multiple of TL), so the first matmul of every
    PSUM accumulation group covers the full tile."""
    plans = {}
    for ci, k in enumerate(KS):
        p = (k - 1) // 2
        taps = sorted(range(k), key=lambda dk: (dk - p != 0, dk))
        for lt in range(L // TL):
            l0 = TL * lt
            mms = []
            for dk in taps:
                o = dk - p
                ins = [_reflect(l0 + j + o) for j in range(TL)]
                # break into maximal runs: consecutive l AND no ring wrap
                runs = []
                j = 0
                while j < TL:
                    j2 = j
                    while (j2 + 1 < TL and ins[j2 + 1] == ins[j2] + 1
                           and ins[j2] % RING != RING - 1):
                        j2 += 1
                    runs.append((j, j2 - j + 1))
                    j = j2 + 1
                for kc in range(HC):
                    for (j, nl) in runs:
                        mms.append((dk, kc, ins[j] % RING, nl, j))
            plans[(lt, ci)] = mms
    return plans


def build_nc(prelu_a: float, rep: int = 1):
    nc = bass.Bass(num_devices=NCORES)

    xt_d = nc.declare_dram_parameter("xt", [NQ, 128, DC, QN], BF16,
                                     isOutput=False)
    wshA_d = nc.declare_dram_parameter("wshA", [SHROWS, CBA], BF16,
                                       isOutput=False)
    wshB_d = nc.declare_dram_parameter("wshB", [SHROWS, CBB], BF16,
                                       isOutput=False)
    bias_d = nc.declare_dram_parameter("bias", [128, 72], F32, isOutput=False)
    out_d = nc.declare_dram_parameter("out", [NC_N, L, C], BF16, isOutput=True)

    plans = _conv_mm_plan()

    with TileContext(nc) as tc:
        with (
            tc.tile_pool(name="dram", bufs=1, space="DRAM") as dpool,
            tc.tile_pool(name="const", bufs=1) as cpool,
            tc.tile_pool(name="main", bufs=1) as mpool,
            tc.tile_pool(name="yt", bufs=2) as ypool,
            tc.tile_pool(name="msk", bufs=1) as kpool,
        ):
            # ---- gather the weight blobs across cores, land them in SBUF ----
            inA = dpool.tile([SHROWS, CBA], BF16, tag="inA", name="inA")
            outA = dpool.tile([128, CBA], BF16, tag="outA", name="outA")
            inB = dpool.tile([SHROWS, CBB], BF16, tag="inB", name="inB")
            outB = dpool.tile([128, CBB], BF16, tag="outB", name="outB")
            nc.gpsimd.dma_start(inA[:], wshA_d[:, :])
            nc.gpsimd.dma_start(inB[:], wshB_d[:, :])
            nc.gpsimd.collective_compute(
                "AllGather", ALU.bypass,
                replica_groups=[list(range(NCORES))],
                ins=[inA.opt()], outs=[outA.opt()])
            nc.gpsimd.collective_compute(
                "AllGather", ALU.bypass,
                replica_groups=[list(range(NCORES))],
                ins=[inB.opt()], outs=[outB.opt()])
            wsbA = cpool.tile([128, CBA], BF16, tag="wsbA", name="wsbA")
            nc.gpsimd.dma_start(out=wsbA[:], in_=outA[:, :])
            wsbB = cpool.tile([128, CBB], BF16, tag="wsbB", name="wsbB")
            nc.gpsimd.dma_start(out=wsbB[:], in_=outB[:, :])
            bias_sb = cpool.tile([128, 72], F32, tag="bias", name="bias")
            nc.sync.dma_start(out=bias_sb[:], in_=bias_d[:, :])

            def bsum_b(m):
                return bias_sb[:, m:m + 1]

            def cb_b(ci):
                return bias_sb[:, 4 + ci:5 + ci]

            def wx_w(d, m):
                c0 = WXO + (d * HC + m) * 128
                return wsbA[:, c0:c0 + 128]

            def wh_w(kc, m):
                c0 = WHO + (kc * HC + m) * 128
                return wsbA[:, c0:c0 + 128]

            def wc_w(k, dk, kc):
                c0 = WCB[k] + (dk * HC + kc) * 128
                return wsbB[:, c0:c0 + 128]

            def wo_w(kc):
                c0 = WOO + kc * C
                return wsbB[:, c0:c0 + C]

            # ---- persistent per-quarter state ----
            hs = [[mpool.tile([128, RING, QN], BF16, tag=f"hs{q}_{m}",
                              name=f"hs{q}_{m}") for m in range(HC)]
                  for q in range(NQ)]
            xw = [[mpool.tile([128, QN], F32, tag=f"xw{q}_{m}",
                              name=f"xw{q}_{m}") for m in range(HC)]
                  for q in range(NQ)]
            P = [mpool.tile([64, L, C], F32, tag=f"P{q}", name=f"P{q}")
                 for q in range(NQ)]
            Po = [mpool.tile([64, L, C], BF16, tag=f"Po{q}", name=f"Po{q}")
                  for q in range(NQ)]
            emax = [kpool.tile([64, L], F32, tag=f"emax{q}", name=f"emax{q}")
                    for q in range(NQ)]
            eend = [kpool.tile([64, L], F32, tag=f"eend{q}", name=f"eend{q}")
                    for q in range(NQ)]
            mkl = [kpool.tile([64, L], F32, tag=f"mkl{q}", name=f"mkl{q}")
                   for q in range(NQ)]
            run = [kpool.tile([64, 1], F32, tag=f"run{q}", name=f"run{q}")
                   for q in range(NQ)]
            bout_bc = bias_sb[0:64, 8:72]

            def rnn_step(q, t, ps1):
                for m in range(HC):
                    if t == 0:
                        nc.scalar.activation(hs[q][m][:, 0, :], xw[q][m][:],
                                             AF.Tanh, bias=bsum_b(m))
                        continue
                    ps = ps1.tile([128, QN], F32, tag="ps1", name="ps1")
                    for kc in range(HC):
                        nc.tensor.matmul(ps[:], wh_w(kc, m),
                                         hs[q][kc][:, (t - 1) % RING, :],
                                         start=(kc == 0), stop=(kc == HC - 1))
                    tmp = ypool.tile([128, QN], F32, tag="rnntmp",
                                     name="rnntmp", bufs=3)
                    nc.vector.tensor_tensor(tmp[:], ps[:], xw[q][m][:],
                                            op=ALU.add)
                    nc.scalar.activation(hs[q][m][:, t % RING, :], tmp[:],
                                         AF.Tanh, bias=bsum_b(m))

            def conv_ltile(q, lt, ps2, ps3):
                l0 = TL * lt
                yts = []
                for ci, k in enumerate(KS):
                    psc = ps2.tile([128, TL, QN], F32, tag="psc", name="psc")
                    mms = plans[(lt, ci)]
                    nmm = len(mms)
                    for idx, (dk, kc, s0, n_l, out_j) in enumerate(mms):
                        dst = psc[:, :, :] if n_l == TL else \
                            psc[:, out_j:out_j + n_l, :]
                        nc.tensor.matmul(
                            dst, wc_w(k, dk, kc),
                            hs[q][kc][:, s0:s0 + n_l, :],
                            start=(idx == 0), stop=(idx == nmm - 1))
                    yt = ypool.tile([128, TL, QN], BF16, tag=f"yt{ci}",
                                    name=f"yt{ci}")
                    nc.scalar.activation(yt[:], psc[:], AF.Prelu,
                                         bias=cb_b(ci),
                                         alpha=float(prelu_a))
                    yts.append(yt)
                for j in range(TL):
                    l = l0 + j
                    psp = ps3.tile([64, C], F32, tag="psp", name="psp")
                    for kc in range(HC):
                        nc.tensor.matmul(
                            psp[:], yts[kc][:, j, :], wo_w(kc),
                            start=(kc == 0), stop=(kc == HC - 1))
                    nc.vector.tensor_tensor(P[q][:, l, :], psp[:],
                                            bout_bc, op=ALU.add)
                    nc.vector.tensor_reduce(
                        emax[q][:, l:l + 1], P[q][:, l:l + 1, 1:],
                        axis=mybir.AxisListType.X, op=ALU.max)
                    nc.vector.tensor_tensor(
                        eend[q][:, l:l + 1], P[q][:, l, 0:1],
                        emax[q][:, l:l + 1], op=ALU.is_ge)
                    nc.vector.tensor_scalar(
                        mkl[q][:, l:l + 1], run[q][:], 0.0, None,
                        ALU.is_equal)
                    nc.vector.tensor_scalar(
                        Po[q][:, l, :], P[q][:, l, :],
                        mkl[q][:, l:l + 1], None, ALU.mult)
                    nc.vector.tensor_tensor(
                        run[q][:], run[q][:], eend[q][:, l:l + 1],
                        op=ALU.max)
                nc.sync.dma_start(
                    out=out_d[QN * q:QN * (q + 1), l0:l0 + TL, :],
                    in_=Po[q][:, l0:l0 + TL, :])

            # conv tile lt needs rnn steps <= TL*lt + TL-1+3
            conv_after = {10: 0, 18: 1, 26: 2, 31: 3}

            for _ in range(rep):
                with (
                    tc.tile_pool(name="ph0", bufs=1) as p0pool,
                    tc.tile_pool(name="ps1", bufs=2, space="PSUM") as ps1,
                    tc.tile_pool(name="ps2", bufs=4, space="PSUM") as ps2,
                    tc.tile_pool(name="ps3", bufs=2, space="PSUM") as ps3,
                ):
                    xq = [p0pool.tile([128, DC, QN], BF16, tag=f"xq{q}",
                                      name=f"xq{q}") for q in range(NQ)]
                    for q in range(NQ):
                        nc.sync.dma_start(out=xq[q][:], in_=xt_d[q, :, :, :])
                    for q in range(NQ):
                        # phase 0: xw = x @ Wx for this quarter
                        for m in range(HC):
                            ps = ps1.tile([128, QN], F32, tag="ps1",
                                          name="ps1")
                            for d in range(DC):
                                nc.tensor.matmul(ps[:], wx_w(d, m),
                                                 xq[q][:, d, :],
                                                 start=(d == 0),
                                                 stop=(d == DC - 1))
                            nc.vector.tensor_copy(xw[q][m][:], ps[:])
                        nc.vector.memset(run[q][:], 0.0)
                        # interleaved RNN + conv + proj + mask + out stream
                        for t in range(L):
                            rnn_step(q, t, ps1)
                            if t in conv_after:
                                conv_ltile(q, conv_after[t], ps2, ps3)

    _split_excess_waits(nc, limit=1)
    return nc


def _pack_inputs(inputs):
    """Host-side packing into PE-ready layouts (bf16 blobs + per-core x)."""
    x = np.ascontiguousarray(inputs["h_w_action"].reshape(N, D_IN))

    blobA = np.empty((128, CBA), dtype=NPBF16)
    wx = inputs["Wx"].reshape(DC, 128, HC, 128).transpose(1, 0, 2, 3)
    blobA[:, WXO:WHO] = wx.reshape(128, 6144).astype(NPBF16)
    wh = inputs["Wh"].reshape(HC, 128, HC, 128).transpose(1, 0, 2, 3)
    blobA[:, WHO:CBA] = wh.reshape(128, 2048).astype(NPBF16)

    blobB = np.empty((128, CBB), dtype=NPBF16)
    for k in KS:
        w = inputs[f"conv_w{k}"]                      # (128, 512, k)
        wt = w.transpose(1, 2, 0).reshape(HC, 128, k, 128)
        wt = wt.transpose(1, 2, 0, 3).reshape(128, k * HC * 128)
        blobB[:, WCB[k]:WCB[k] + k * 512] = wt.astype(NPBF16)
    wo = inputs["Wout"].reshape(HC, 128, C).transpose(1, 0, 2)
    blobB[:, WOO:CBB] = wo.reshape(128, HC * C).astype(NPBF16)

    bias = np.empty((128, 72), dtype=np.float32)
    bias[:, 0:4] = (inputs["bx"] + inputs["bh"]).reshape(HC, 128).T
    bias[:, 4:8] = np.concatenate(
        [inputs[f"conv_b{k}"] for k in KS]).reshape(HC, 128).T
    bias[:, 8:72] = np.broadcast_to(inputs["bout"].reshape(1, C), (128, C))

    in_maps = []
    for c in range(NCORES):
        xs = x[c * NC_N:(c + 1) * NC_N]               # (256, 1536)
        xt = xs.T.reshape(DC, 128, NC_N).transpose(1, 0, 2)   # [128, DC, 256]
        xt4 = np.ascontiguousarray(
            xt.reshape(128, DC, NQ, QN).transpose(2, 0, 1, 3)).astype(NPBF16)
        m = {"xt": xt4,
             "wshA": np.ascontiguousarray(blobA[c * SHROWS:(c + 1) * SHROWS]),
             "wshB": np.ascontiguousarray(blobB[c * SHROWS:(c + 1) * SHROWS]),
             "bias": bias}
        in_maps.append(m)
    return in_maps


_NC_CACHE = {}
_RUNNER_CACHE = {}


def _make_runner(nc):
    """Persistent jitted PJRT runner (mirrors bass2jax.run_bass_via_pjrt's
    multi-core path) so repeat kernel() calls skip re-tracing."""
    import jax
    from jax.sharding import Mesh, PartitionSpec
    try:
        from jax.experimental.shard_map import shard_map
    except ImportError:
        from jax import shard_map
    from concourse import bass2jax

    bass2jax.install_neuronx_cc_hook()
    partition_name = (nc.partition_id_tensor.name
                      if nc.partition_id_tensor else None)
    in_names, out_names, out_avals, zero_outs = [], [], [], []
    for alloc in nc.m.functions[0].allocations:
        if not isinstance(alloc, mybir.MemoryLocationSet):
            continue
        name = alloc.memorylocations[0].name
        if alloc.kind == "ExternalInput":
            if name != partition_name:
                in_names.append(name)
        elif alloc.kind == "ExternalOutput":
            shape = tuple(alloc.tensor_shape)
            dtype = mybir.dt.np(alloc.dtype)
            out_names.append(name)
            out_avals.append(jax.core.ShapedArray(shape, dtype))
            zero_outs.append(np.zeros(shape, dtype))
    n_params, n_outs = len(in_names), len(out_avals)
    all_in_names = list(in_names) + list(out_names)
    if partition_name is not None:
        all_in_names.append(partition_name)

    def _body(*args):
        operands = list(args)
        if partition_name is not None:
            operands.append(bass2jax.partition_id_tensor())
        return tuple(bass2jax._bass_exec_p.bind(
            *operands,
            out_avals=tuple(out_avals),
            in_names=tuple(all_in_names),
            out_names=tuple(out_names),
            lowering_input_output_aliases=(),
            sim_require_finite=True,
            sim_require_nnan=True,
            nc=nc,
        ))

    devices = jax.devices()[:NCORES]
    mesh = Mesh(np.asarray(devices), ("core",))
    in_specs = (PartitionSpec("core"),) * (n_params + n_outs)
    out_specs = (PartitionSpec("core"),) * n_outs
    donate = tuple(range(n_params, n_params + n_outs))
    sharded = jax.jit(
        shard_map(_body, mesh=mesh, in_specs=in_specs, out_specs=out_specs,
                  check_rep=False),
        donate_argnums=donate, keep_unused=True)

    def call(in_maps):
        concat_in = [np.concatenate([np.asarray(in_maps[c][nm])
                                     for c in range(NCORES)], axis=0)
                     for nm in in_names]
        zeros = [np.zeros((NCORES * z.shape[0], *z.shape[1:]), z.dtype)
                 for z in zero_outs]
        out_arrs = sharded(*concat_in, *zeros)
        oidx = out_names.index("out")
        full = np.asarray(out_arrs[oidx])
        return full.reshape(NCORES, NC_N, L, C)

    return call


def kernel(**inputs) -> np.ndarray:
    inputs = {k: np.asarray(v, dtype=np.float32) for k, v in inputs.items()}
    prelu_a = float(np.asarray(inputs["prelu_a"]))
    key = (prelu_a, 1)
    if key not in _NC_CACHE:
        _NC_CACHE[key] = build_nc(prelu_a, rep=1)
    nc = _NC_CACHE[key]
    in_maps = _pack_inputs(inputs)
    try:
        if key not in _RUNNER_CACHE:
            _RUNNER_CACHE[key] = _make_runner(nc)
        out = _RUNNER_CACHE[key](in_maps)
    except Exception:
        res = run_bass_kernel_spmd(nc, in_maps, core_ids=list(range(NCORES)))
        out = np.stack([res.results[c]["out"] for c in range(NCORES)], axis=0)
    return out.reshape(E, S, L, C).astype(np.float32)
